# revision 5
# baseline (speedup 1.0000x reference)
"""GCN (4-layer, PyG-default GCNConv) forward on 8 Trainium2 NeuronCores.

Strategy (node-parallel / graph-parallel):
  - Nodes are partitioned contiguously across the 8 cores (1250 rows each).
  - Per layer: each core computes its row-slice of G = H @ W as a tiled PE
    GEMM (bf16 in / fp32 accumulate), the slices are AllGathered into a
    replicated HBM copy, and each core aggregates its own destination rows:
    messages are fetched with dma_gather (one gather per 128-dst-node block)
    and summed on the PE as OUT_block = S_block.T @ MSG, where S_block is a
    host-precomputed dense [128-edge-chunk, 128-dst] matrix holding the
    symmetric-normalization weight of each edge (zero elsewhere).
  - Layer 4 is reassociated as (A_hat @ H4) @ W4 + b4 so the aggregation
    stays 512 wide and the tiny final GEMM runs in fp32.
  - log_softmax over the 2 classes is fused on-chip.

The graph structure (edge_index) only affects host-side preprocessing
(sorting edges by destination, building S blocks and gather indices);
the weights of every layer reuse the same structures.
"""

import sys

sys.path.insert(0, "/opt/trn_rl_repo")

import numpy as np
import ml_dtypes

BF16 = ml_dtypes.bfloat16

# Problem constants (nn_GCN_39195871543847)
N, E, F_IN, HID, C = 10000, 160000, 2208, 512, 2
W_CORES = 8
RPC = N // W_CORES  # 1250 nodes per core
MB = 10  # 128-row blocks per core (10 * 128 = 1280 padded rows)
RPAD = MB * 128  # 1280
KFC = (F_IN + 127) // 128  # 18 contraction chunks for layer 1 (pad 2208->2304)
KFP = KFC * 128  # 2304
GROWS = W_CORES * RPAD  # 10240 rows in the replicated gathered tensor
C_PAD = 64  # pad 2 output classes to 64 fp32 (256B rows)
G_CHUNKS = 2  # 128-index chunks per dma_gather call (>2 crashes this ucode)
N_QUEUES = 4  # SWDGE queues to spread descriptor generation across Q7 pairs


def _install_drain_patch():
    """This container's walrus accepts at most one sync-wait per instruction;
    TileContext's final drain gets one wait per live semaphore. Split the
    extra waits onto single-wait NOPs."""
    import bass_rust
    import concourse.tile as tile
    from concourse.vector_clock import ScopedClock

    if getattr(tile.TileContext, "_drain_patch_installed", False):
        return

    def _drain_and_barrier(self, tick_clock, wait_clock):
        drain_inst = self.nc.sync.drain()
        wait_clock.add_sem_waits(
            drain_inst.ins, ScopedClock({None: tick_clock.global_clock})
        )
        si = drain_inst.ins.sync_info
        waits = list(si.on_wait or []) if si is not None else []
        if len(waits) > 1:
            si.on_wait = waits[:1]
            for w in waits[1:]:
                nop = self.nc.sync.nop(nofuse=True)
                nop.ins.sync_info = bass_rust.SyncInfo(on_wait=[w], on_update=[])
        self.nc.all_engine_barrier()
        assert self.sems is not None
        popped = self.nc._tile_sem_poison_stack.pop()
        assert popped is self._sem_poison
        self.nc.clear_and_free_semaphores(list(self.sems.allocated().values()))
        self.nc.all_engine_barrier()

    tile.TileContext._drain_and_barrier = _drain_and_barrier
    tile.TileContext._drain_patch_installed = True


# ----------------------------------------------------------------------------
# Host-side graph preprocessing
# ----------------------------------------------------------------------------


def _preprocess(edge_index):
    """Build, per core: S blocks [T,128,128] (bf16 edge weights), gather
    index layout [128, T*8] (int16), where T = MB * k_fix chunks."""
    src = edge_index[0].astype(np.int64)
    dst = edge_index[1].astype(np.int64)
    loop = np.arange(N, dtype=np.int64)
    s = np.concatenate([src, loop])
    d = np.concatenate([dst, loop])
    deg = np.bincount(d, minlength=N).astype(np.float32)
    dinv = np.where(deg > 0, 1.0 / np.sqrt(deg), 0.0).astype(np.float32)
    norm = dinv[s] * dinv[d]

    core = d // RPC
    per_core = []
    k_fix = 1
    for c in range(W_CORES):
        m = core == c
        sc, dc, wc = s[m], d[m] - c * RPC, norm[m]
        order = np.argsort(dc, kind="stable")
        sc, dc, wc = sc[order], dc[order], wc[order]
        blk = dc // 128
        mloc = dc % 128
        counts = np.bincount(blk, minlength=MB)
        k_fix = max(k_fix, int(np.max((counts + 127) // 128)))
        per_core.append((sc, blk, mloc, wc, counts))

    T = MB * k_fix
    s_blocks = []
    idx_layouts = []
    for sc, blk, mloc, wc, counts in per_core:
        starts = np.zeros(MB, np.int64)
        starts[1:] = np.cumsum(counts)[:-1]
        pos = np.arange(len(sc)) - starts[blk]  # position within block
        u = pos // 128
        k = pos % 128
        t = blk * k_fix + u
        S = np.zeros((T, 128, 128), np.float32)
        S[t, k, mloc] = wc
        # gather row id in the all-gathered [GROWS, .] tensor
        g_idx = (sc // RPC) * RPAD + (sc % RPC)
        idx_flat = np.zeros(T * 128, np.int16)
        idx_flat[t * 128 + k] = g_idx.astype(np.int16)
        # index i -> partition i%16, column i//16, replicated over 8 groups
        lay16 = idx_flat.reshape(T * 8, 16).T  # [16, T*8]
        idx_layouts.append(np.tile(lay16, (8, 1)).astype(np.int16))
        s_blocks.append(S.astype(BF16))
    return k_fix, s_blocks, idx_layouts


def _prep_inputs(x, edge_index, W1, b1, W2, b2, W3, b3, W4, b4):
    k_fix, s_blocks, idx_layouts = _preprocess(edge_index)

    # xT per core: [MB, KFC, 128, 128] bf16, xT[m,k,p,j] = x[c*RPC + m*128+j, k*128+p]
    xts = []
    for c in range(W_CORES):
        xp = np.zeros((RPAD, KFP), np.float32)
        xp[:RPC, :F_IN] = x[c * RPC : (c + 1) * RPC]
        xt = xp.reshape(MB, 128, KFC, 128).transpose(0, 2, 3, 1)
        xts.append(np.ascontiguousarray(xt).astype(BF16))

    W1p = np.zeros((KFP, HID), np.float32)
    W1p[:F_IN] = W1
    W1l = np.ascontiguousarray(W1p.reshape(KFC, 128, HID)).astype(BF16)
    W2l = np.ascontiguousarray(W2.reshape(4, 128, HID)).astype(BF16)
    W3l = np.ascontiguousarray(W3.reshape(4, 128, HID)).astype(BF16)
    W4p = np.zeros((HID, C_PAD), np.float32)
    W4p[:, :C] = W4
    W4l = np.ascontiguousarray(W4p.reshape(4, 128, C_PAD)).astype(np.float32)

    b1r = np.broadcast_to(b1, (128, HID)).astype(np.float32).copy()
    b2r = np.broadcast_to(b2, (128, HID)).astype(np.float32).copy()
    b3r = np.broadcast_to(b3, (128, HID)).astype(np.float32).copy()
    b4r = np.zeros((128, C_PAD), np.float32)
    b4r[:, :C] = b4

    in_maps = []
    for c in range(W_CORES):
        in_maps.append(
            {
                "xT": xts[c],
                "W1l": W1l,
                "W2l": W2l,
                "W3l": W3l,
                "W4l": W4l,
                "b1r": b1r,
                "b2r": b2r,
                "b3r": b3r,
                "b4r": b4r,
                "S_in": s_blocks[c],
                "idx_in": idx_layouts[c],
            }
        )
    return k_fix, in_maps


# ----------------------------------------------------------------------------
# Bass kernel builder
# ----------------------------------------------------------------------------

_cache = {}


def _build(k_fix):
    import concourse.bass as bass
    import concourse.mybir as mybir
    from concourse.bacc import Bacc
    from concourse.tile import TileContext
    from concourse.masks import make_identity

    f32 = mybir.dt.float32
    bf16 = mybir.dt.bfloat16
    i16 = mybir.dt.int16
    T = MB * k_fix

    nc = Bacc(num_devices=W_CORES, num_swdge_queues=N_QUEUES)
    gq = [0]  # round-robin cursor for gather queues

    xT = nc.dram_tensor("xT", [MB, KFC, 128, 128], bf16, kind="ExternalInput")
    W1l = nc.dram_tensor("W1l", [KFC, 128, HID], bf16, kind="ExternalInput")
    W2l = nc.dram_tensor("W2l", [4, 128, HID], bf16, kind="ExternalInput")
    W3l = nc.dram_tensor("W3l", [4, 128, HID], bf16, kind="ExternalInput")
    W4l = nc.dram_tensor("W4l", [4, 128, C_PAD], f32, kind="ExternalInput")
    b1r = nc.dram_tensor("b1r", [128, HID], f32, kind="ExternalInput")
    b2r = nc.dram_tensor("b2r", [128, HID], f32, kind="ExternalInput")
    b3r = nc.dram_tensor("b3r", [128, HID], f32, kind="ExternalInput")
    b4r = nc.dram_tensor("b4r", [128, C_PAD], f32, kind="ExternalInput")
    S_in = nc.dram_tensor("S_in", [T, 128, 128], bf16, kind="ExternalInput")
    idx_in = nc.dram_tensor("idx_in", [128, T * 8], i16, kind="ExternalInput")
    out = nc.dram_tensor("out", [RPAD, C], f32, kind="ExternalOutput")

    g_own = [
        nc.dram_tensor(f"g_own{l}", [RPAD, HID], bf16, kind="Internal")
        for l in range(3)
    ]
    h4_own = nc.dram_tensor("h4_own", [RPAD, HID], bf16, kind="Internal")
    g_full = [
        nc.dram_tensor(
            f"g_full{l}", [GROWS, HID], bf16, kind="Internal", addr_space="Shared"
        )
        for l in range(3)
    ]
    h4_full = nc.dram_tensor(
        "h4_full", [GROWS, HID], bf16, kind="Internal", addr_space="Shared"
    )

    rg = [list(range(W_CORES))]

    with TileContext(nc) as tc:
        with (
            tc.tile_pool(name="const", bufs=1) as cpool,
            tc.tile_pool(name="work", bufs=2) as wpool,
            tc.tile_pool(name="psum", bufs=2, space="PSUM") as ppool,
        ):
            # ---- resident tensors -------------------------------------------------
            S_sb = cpool.tile([128, T, 128], bf16)
            nc.sync.dma_start(out=S_sb[:], in_=S_in.rearrange("t k m -> k t m"))
            idx_sb = cpool.tile([128, T * 8], i16)
            nc.sync.dma_start(out=idx_sb[:], in_=idx_in[:])
            W1_sb = cpool.tile([128, KFC, HID], bf16)
            nc.sync.dma_start(out=W1_sb[:], in_=W1l.rearrange("c k h -> k c h"))
            W2_sb = cpool.tile([128, 4, HID], bf16)
            nc.sync.dma_start(out=W2_sb[:], in_=W2l.rearrange("c k h -> k c h"))
            W3_sb = cpool.tile([128, 4, HID], bf16)
            nc.sync.dma_start(out=W3_sb[:], in_=W3l.rearrange("c k h -> k c h"))
            W4_sb = cpool.tile([128, 4, C_PAD], f32)
            nc.sync.dma_start(out=W4_sb[:], in_=W4l.rearrange("c k h -> k c h"))
            b_sb = []
            for nm, src in (("b1", b1r), ("b2", b2r), ("b3", b3r)):
                t = cpool.tile([128, HID], f32, tag=f"bias_{nm}")
                nc.sync.dma_start(out=t[:], in_=src[:])
                b_sb.append(t)
            b4_sb = cpool.tile([128, C_PAD], f32)
            nc.sync.dma_start(out=b4_sb[:], in_=b4r[:])
            id_bf = cpool.tile([128, 128], bf16)
            make_identity(nc, id_bf[:])
            id_f32 = cpool.tile([128, 128], f32)
            make_identity(nc, id_f32[:])

            relu = mybir.ActivationFunctionType.Relu

            def gemm_layer(ht_tiles, w_sb, nk, dst_dram):
                """G_own = H @ W from transposed H tiles; cast bf16, DMA out."""
                dmas = []
                for m in range(MB):
                    ps = ppool.tile([128, HID], f32, tag="gps")
                    for k in range(nk):
                        nc.tensor.matmul(
                            ps[:],
                            lhsT=ht_tiles[m][:, k, :],
                            rhs=w_sb[:, k, :],
                            start=(k == 0),
                            stop=(k == nk - 1),
                        )
                    gb = wpool.tile([128, HID], bf16, tag="gb")
                    nc.scalar.copy(gb[:], ps[:])
                    dmas.append(
                        nc.sync.dma_start(
                            out=dst_dram[m * 128 : (m + 1) * 128, :], in_=gb[:]
                        )
                    )
                return dmas

            def aggregate(src_full, bias_t, do_relu, ht_dtype, ht_tag, ht_bufs,
                          store_h_dram=None):
                """For each dst block: gather messages, S.T @ MSG on PE,
                (+bias, relu), transpose into [128, 4, 128] tiles."""
                ht_tiles = []
                h_dmas = []
                for b in range(MB):
                    ps = ppool.tile([128, HID], f32, tag="aps")
                    for g0 in range(0, k_fix, G_CHUNKS):
                        ngc = min(G_CHUNKS, k_fix - g0)
                        t0 = b * k_fix + g0
                        msg = wpool.tile(
                            [128, G_CHUNKS, HID], bf16, tag="msg", bufs=8
                        )
                        nc.gpsimd.dma_gather(
                            out_ap=msg[:, :ngc, :],
                            in_ap=src_full[:],
                            idxs_ap=idx_sb[:, t0 * 8 : (t0 + ngc) * 8],
                            num_idxs=ngc * 128,
                            num_idxs_reg=ngc * 128,
                            elem_size=HID,
                            queue_num=gq[0],
                        )
                        gq[0] = (gq[0] + 1) % N_QUEUES
                        for u in range(ngc):
                            nc.tensor.matmul(
                                ps[:],
                                lhsT=S_sb[:, t0 + u, :],
                                rhs=msg[:, u, :],
                                start=(g0 == 0 and u == 0),
                                stop=(g0 + u == k_fix - 1),
                            )
                    if bias_t is not None:
                        hf = wpool.tile([128, HID], f32, tag="hf")
                        nc.vector.tensor_add(out=hf[:], in0=ps[:], in1=bias_t[:])
                        hsrc = hf
                    else:
                        hsrc = ps
                    hb = wpool.tile([128, HID], ht_dtype, tag=f"hb_{ht_tag}")
                    if do_relu:
                        nc.scalar.activation(hb[:], hsrc[:], relu)
                    else:
                        nc.scalar.copy(hb[:], hsrc[:])
                    if store_h_dram is not None:
                        h_dmas.append(
                            nc.sync.dma_start(
                                out=store_h_dram[b * 128 : (b + 1) * 128, :],
                                in_=hb[:],
                            )
                        )
                    ident = id_bf if ht_dtype == bf16 else id_f32
                    ht = wpool.tile([128, 4, 128], ht_dtype, tag=ht_tag, bufs=ht_bufs)
                    for g in range(4):
                        tp = ppool.tile([128, 128], ht_dtype, tag="tps")
                        nc.tensor.transpose(
                            tp[:], hb[:, g * 128 : (g + 1) * 128], ident[:]
                        )
                        nc.vector.tensor_copy(out=ht[:, g, :], in_=tp[:])
                    ht_tiles.append(ht)
                return ht_tiles, h_dmas

            def allgather(own, full):
                nc.gpsimd.collective_compute(
                    "AllGather",
                    mybir.AluOpType.bypass,
                    ins=[own[:]],
                    outs=[full[:]],
                    replica_groups=rg,
                )

            # ---- layer 1 GEMM: X @ W1 --------------------------------------------
            for m in range(MB):
                xm = wpool.tile([128, KFC, 128], bf16, tag="xm")
                nc.sync.dma_start(
                    out=xm[:], in_=xT[m].rearrange("c p j -> p c j")
                )
                ps = ppool.tile([128, HID], f32, tag="gps")
                for k in range(KFC):
                    nc.tensor.matmul(
                        ps[:],
                        lhsT=xm[:, k, :],
                        rhs=W1_sb[:, k, :],
                        start=(k == 0),
                        stop=(k == KFC - 1),
                    )
                gb = wpool.tile([128, HID], bf16, tag="gb")
                nc.scalar.copy(gb[:], ps[:])
                nc.sync.dma_start(out=g_own[0][m * 128 : (m + 1) * 128, :], in_=gb[:])

            allgather(g_own[0], g_full[0])

            # ---- layers 1..3 aggregation + next GEMM -----------------------------
            ht2, _ = aggregate(g_full[0], b_sb[0], True, bf16, "ht", 12)
            gemm_layer(ht2, W2_sb, 4, g_own[1])
            allgather(g_own[1], g_full[1])

            ht3, _ = aggregate(g_full[1], b_sb[1], True, bf16, "ht", 12)
            gemm_layer(ht3, W3_sb, 4, g_own[2])
            allgather(g_own[2], g_full[2])

            # layer-3 aggregation produces H4; store it for the all-gather
            _, _ = aggregate(g_full[2], b_sb[2], True, bf16, "ht", 12,
                             store_h_dram=h4_own)
            allgather(h4_own, h4_full)

            # ---- layer 4: Z = A_hat @ H4 (fp32 ZT), then Z @ W4 + b4, log_softmax
            zt, _ = aggregate(h4_full, None, False, mybir.dt.float32, "zt", 10)

            for m in range(MB):
                ps = ppool.tile([128, C_PAD], f32, tag="gps")
                for k in range(4):
                    nc.tensor.matmul(
                        ps[:],
                        lhsT=zt[m][:, k, :],
                        rhs=W4_sb[:, k, :],
                        start=(k == 0),
                        stop=(k == 3),
                    )
                lg = wpool.tile([128, C_PAD], f32, tag="lg")
                nc.vector.tensor_add(out=lg[:], in0=ps[:], in1=b4_sb[:])
                mx = wpool.tile([128, 1], f32, tag="mx")
                nc.vector.tensor_reduce(
                    out=mx[:], in_=lg[:, :C], axis=mybir.AxisListType.X,
                    op=mybir.AluOpType.max,
                )
                t2 = wpool.tile([128, C], f32, tag="t2")
                nc.vector.tensor_scalar(
                    out=t2[:], in0=lg[:, :C], scalar1=mx[:], scalar2=None,
                    op0=mybir.AluOpType.subtract,
                )
                e2 = wpool.tile([128, C], f32, tag="e2")
                nc.scalar.activation(e2[:], t2[:], mybir.ActivationFunctionType.Exp)
                sm = wpool.tile([128, 1], f32, tag="sm")
                nc.vector.tensor_reduce(
                    out=sm[:], in_=e2[:], axis=mybir.AxisListType.X,
                    op=mybir.AluOpType.add,
                )
                ls = wpool.tile([128, 1], f32, tag="ls")
                nc.scalar.activation(ls[:], sm[:], mybir.ActivationFunctionType.Ln)
                o2 = wpool.tile([128, C], f32, tag="o2")
                nc.vector.tensor_scalar(
                    out=o2[:], in0=t2[:], scalar1=ls[:], scalar2=None,
                    op0=mybir.AluOpType.subtract,
                )
                nc.sync.dma_start(out=out[m * 128 : (m + 1) * 128, :], in_=o2[:])

    nc.compile()
    return nc


# ----------------------------------------------------------------------------
# Entry point
# ----------------------------------------------------------------------------


def kernel(x, edge_index, batch, W1, b1, W2, b2, W3, b3, W4, b4, _trace=False):
    _install_drain_patch()
    from concourse.bass_utils import run_bass_kernel_spmd

    x = np.asarray(x, np.float32)
    edge_index = np.asarray(edge_index)
    k_fix, in_maps = _prep_inputs(
        np.asarray(x, np.float32),
        edge_index,
        np.asarray(W1, np.float32), np.asarray(b1, np.float32),
        np.asarray(W2, np.float32), np.asarray(b2, np.float32),
        np.asarray(W3, np.float32), np.asarray(b3, np.float32),
        np.asarray(W4, np.float32), np.asarray(b4, np.float32),
    )
    if k_fix not in _cache:
        _cache[k_fix] = _build(k_fix)
    nc = _cache[k_fix]
    res = run_bass_kernel_spmd(
        nc, in_maps, core_ids=list(range(W_CORES)), trace=_trace
    )
    outp = np.concatenate(
        [res.results[c]["out"][:RPC] for c in range(W_CORES)], axis=0
    ).astype(np.float32)
    if _trace:
        return outp, res
    return outp
